# revision 3
# baseline (speedup 1.0000x reference)
"""BuzzLoss Trainium2 kernel — single fused custom-DVE op per tile.

Math (telescoped form of the reference):
    excl[t] = prod_{s<t} (1 - conf[s])          (exclusive cumprod)
    score_b = sum_t excl[b,t] * da[b,t]
    da[b,0] = acc[b,0];  da[b,t] = acc[b,t] - acc[b,t-1]
    out = -mean_b score_b

With k = t-1 this is  score_b = acc[b,0] + sum_{k=0}^{T-2} incl[k] * da[k+1]
where incl[k] = prod_{j<=k} nb[j], nb = 1 - conf.  The whole inner sum is one
custom-DVE instruction per 128-row tile:

    Spec(body=scan(MULT, Src0) * Src1, accum=add)
      accum_out[p] = sum_k (prod_{j<=k} Src0[p,j]) * Src1[p,k]

The scan combine uses same-stage CURR_ALU_OUT feedback (no pipeline bubble),
so the op streams at 1 elem/cycle/lane with an fp32 recurrence state — vs the
stock tensor_tensor_scan (half rate) + separate multiply-accumulate pass.

Host prep (dtype/layout only — all reduction work stays on device):
    nbuzz = bf16(1 - conf[:, :T-1]), padded to T cols      (2 MiB/core)
    dash  = int8(acc[:, 1:] - acc[:, :-1]), padded with 0  (1 MiB/core)
The t=0 boundary term (= acc[b,0]) and the final mean are host-side, as is
the cross-core reduction (pure data parallel, batch 8192 = 8 x 1024 rows).

DMA: 3 MiB/core on the SP HWDGE ring (vs 8 MiB fp32 baseline), per-tile
transfers interleaved nb/dash so compute starts after the first pair lands.
"""

import operator

import numpy as np
import ml_dtypes

import concourse.bacc as bacc
import concourse.mybir as mybir
import concourse.tile as tile
import concourse.dve_ops as dve_ops
from concourse.bass_utils import run_bass_kernel_spmd
from concourse.dve_spec import Spec, scan, Src0, Src1, AluOp, lower, _has_src1
from concourse.dve_uop import DveOpSpec

B, T = 8192, 1024
N_CORES = 8
ROWS = B // N_CORES  # rows per core
P = 128  # SBUF partitions
NTILES = ROWS // P  # row-tiles per core

f32 = mybir.dt.float32
bf16 = mybir.dt.bfloat16
i8 = mybir.dt.int8

_OP_NAME = "BUZZ_CUMPROD_MUL_REDUCE"


def _op_reference(in0, in1, c0, c1, c2):
    x = (
        np.cumprod(np.asarray(in0, np.float32), axis=-1)
        * np.asarray(in1, np.float32)
    ).astype(np.float32)
    return x, x.reshape(x.shape[0], -1).sum(axis=-1, keepdims=True).astype(np.float32)


_SPEC = Spec(
    body=scan(AluOp.MULTIPLY, Src0) * Src1,
    accum=operator.add,
    reference=_op_reference,
)


def _register_op() -> "dve_ops.DveOp":
    for op in dve_ops.OPS:
        if op.name == _OP_NAME:
            return op
    row = max(dve_ops._SUB_OPCODE_FOR_NAME.values()) + 1
    assert row < 0x20, "no free custom-DVE opcode row"
    dve_ops._SUB_OPCODE_FOR_NAME[_OP_NAME] = row
    shas = {
        ver: DveOpSpec(
            name=_OP_NAME,
            opcode=row,
            uops=lower(_SPEC, ver=ver),
            rd1_en=_has_src1(_SPEC),
        ).sha(ver)
        for ver in ("v3",)
    }
    op = dve_ops.DveOp(name=_OP_NAME, spec=_SPEC, subdim=False, uops_sha=shas)
    dve_ops.OPS.append(op)
    dve_ops.CUSTOM_DVE_SPECS[_OP_NAME] = _SPEC
    return op


_CACHE = {}


def _emit_pipeline(nc, op, io_pool, work_pool, res, nb_r, da_r, rep):
    nbt, dat = {}, {}
    for j in range(NTILES):
        nbt[j] = io_pool.tile([P, T], bf16, tag="nb", name=f"nb_t{rep}_{j}")
        nc.sync.dma_start(nbt[j][:], nb_r[j])
        dat[j] = io_pool.tile([P, T], i8, tag="da", name=f"da_t{rep}_{j}")
        nc.sync.dma_start(dat[j][:], da_r[j])
    for j in range(NTILES):
        scr = work_pool.tile([P, T], bf16, tag="scr")
        nc.vector._custom_dve(
            op,
            out=scr[:],
            in0=nbt[j][:],
            in1=dat[j][:],
            accum_out=res[:, j : j + 1],
        )


def build_bass(reps: int = 1):
    op = _register_op()
    nc = bacc.Bacc("TRN2", target_bir_lowering=False, debug=False)
    nb = nc.declare_dram_parameter("nbuzz", [ROWS, T], bf16, isOutput=False)
    da = nc.declare_dram_parameter("dash", [ROWS, T], i8, isOutput=False)
    out = nc.declare_dram_parameter("partials", [P, NTILES], f32, isOutput=True)

    nb_r = nb.rearrange("(n p) t -> n p t", p=P)
    da_r = da.rearrange("(n p) t -> n p t", p=P)

    with tile.TileContext(nc) as tc:
        with (
            tc.tile_pool(name="io", bufs=NTILES) as io_pool,
            tc.tile_pool(name="work", bufs=2) as work_pool,
            tc.tile_pool(name="res", bufs=1) as res_pool,
        ):
            res = res_pool.tile([P, NTILES], f32)
            for rep in range(reps):
                _emit_pipeline(nc, op, io_pool, work_pool, res, nb_r, da_r, rep)
            nc.sync.dma_start(out[:], res[:])
    nc.compile()
    return nc


def make_in_maps(confidences: np.ndarray, accuracies: np.ndarray):
    conf = np.asarray(confidences, dtype=np.float32)
    acc = np.asarray(accuracies, dtype=np.float32)
    nb = np.ones((B, T), np.float32)
    np.subtract(1.0, conf[:, : T - 1], out=nb[:, : T - 1])
    nbb = nb.astype(ml_dtypes.bfloat16)
    dash = np.zeros((B, T), np.int8)
    dash[:, : T - 1] = (acc[:, 1:] - acc[:, : T - 1]).astype(np.int8)
    return [
        {
            "nbuzz": nbb[i * ROWS : (i + 1) * ROWS],
            "dash": dash[i * ROWS : (i + 1) * ROWS],
        }
        for i in range(N_CORES)
    ]


def reduce_partials(results, accuracies) -> np.ndarray:
    # device partials + the t=0 boundary term sum_b acc[b, 0]
    total = float(np.sum(np.asarray(accuracies)[:, 0], dtype=np.float64))
    for r in results:
        total += float(np.sum(r["partials"].astype(np.float64)))
    return np.asarray(-(total / B), dtype=np.float32)


def _run_device(confidences: np.ndarray, accuracies: np.ndarray):
    if "nc" not in _CACHE:
        _CACHE["nc"] = build_bass()
    return run_bass_kernel_spmd(
        _CACHE["nc"], make_in_maps(confidences, accuracies), list(range(N_CORES))
    ).results


_CHILD_CODE = """
import sys, numpy as np
sys.path.insert(0, sys.argv[1])
import kernel as K
d = np.load(sys.argv[2])
res = K._run_device(d["confidences"], d["accuracies"])
np.savez(sys.argv[3], **{f"p{i}": r["partials"] for i, r in enumerate(res)})
"""


def _run_subprocess(confidences: np.ndarray, accuracies: np.ndarray):
    # Fresh process -> fresh PJRT client; recovers from a transient
    # device-unrecoverable left by a prior NEFF load (NEFF compile is
    # disk-cached, so the retry costs seconds).
    import os
    import subprocess
    import sys
    import tempfile

    here = os.path.dirname(os.path.abspath(__file__))
    with tempfile.TemporaryDirectory() as td:
        in_path = os.path.join(td, "in.npz")
        out_path = os.path.join(td, "out.npz")
        np.savez(in_path, confidences=confidences, accuracies=accuracies)
        subprocess.run(
            [sys.executable, "-c", _CHILD_CODE, here, in_path, out_path],
            check=True,
            timeout=900,
        )
        d = np.load(out_path)
        return [{"partials": d[f"p{i}"]} for i in range(N_CORES)]


def kernel(confidences: np.ndarray, accuracies: np.ndarray) -> np.ndarray:
    import time

    results = None
    try:
        results = _run_device(confidences, accuracies)
    except Exception:
        for attempt in range(3):
            time.sleep(2.0)
            try:
                results = _run_subprocess(confidences, accuracies)
                break
            except Exception:
                if attempt == 2:
                    raise
    return reduce_partials(results, accuracies)
